# revision 10
# baseline (speedup 1.0000x reference)
"""Trainium2 Bass kernel for the disentangled non-local block.

Per batch b (one NeuronCore each, 8 batches over 8 cores):
  x:  [64, 4096]   (Cin x N, N = 64*64)
  q/k/v = 1x1 conv (64x64 GEMM + bias)
  q' = q - mean_n(q), k' = k - mean_n(k)
  pairwise: out_p[c,n] = sum_m softmax_m(q'_n . k'_m / 8) * v[c,m]
  unary:    out_u[c]   = sum_m softmax_m(q_mean . k'_m) * v[c,m]
  out = x + out_p + out_u

Layout strategy (per core):
  - q' held as [C, N+1] in SBUF; extra column N holds 8*q_mean so the
    unary term is just one more "query" through the same machinery
    (logits get scaled by 1/8, so 8*q_mean . k' / 8 = q_mean . k').
  - S chunks computed with m on partitions: S[m(128), n(1024)] =
    matmul(lhsT=k'[:, mchunk], rhs=q'[:, nblock]).  exp on ScalarE
    (no max subtraction: logits are in [-3, 3] by construction).
  - v held transposed ([N, C] chunks) with a ones column appended, so
    the second matmul O[c(65), n] += vT_chunk.T @ E accumulates both
    the numerator (rows 0..63) and the softmax denominator (row 64).
  - denominator reciprocal broadcast across partitions via GPSIMD
    partition_broadcast; final combine on VectorE; residual add of
    (x + out_u) precomputed once.
"""

import numpy as np

B = 8
CIN = 64
C = 64
H = W = 64
N = H * W            # 4096
NB = 1024            # n-block (columns per outer iteration)
NBLK = N // NB       # 4
MB = 128             # m-chunk (keys per matmul, partition dim)
MCH = N // MB        # 32
HALF = 512           # PSUM bank free-dim for fp32 matmul
SCALE = 0.125        # 1 / (sqrt(C) * temperature)

_CACHE = {}


def _build(repeat=1, compat=True):
    import concourse.bass as bass
    import concourse.tile as tile
    from concourse import mybir

    f32 = mybir.dt.float32
    AX = mybir.AxisListType
    AF = mybir.ActivationFunctionType

    nc = bass.Bass()
    x_d = nc.dram_tensor("x_aug", [CIN + 1, N], f32, kind="ExternalInput")
    wq_d = nc.dram_tensor("wqT_aug", [CIN + 1, C], f32, kind="ExternalInput")
    wk_d = nc.dram_tensor("wkT_aug", [CIN + 1, C], f32, kind="ExternalInput")
    wv_d = nc.dram_tensor("wvT_aug", [CIN + 1, C + 1], f32, kind="ExternalInput")
    out_d = nc.dram_tensor("out", [C, N], f32, kind="ExternalOutput")

    with tile.TileContext(nc) as tc:
        with (
            tc.tile_pool(name="sing", bufs=1) as sing,
            tc.tile_pool(name="epool", bufs=4) as epool,
            tc.tile_pool(name="opool", bufs=2) as opool,
            tc.tile_pool(name="dpool", bufs=2) as dpool,
            tc.tile_pool(name="psA", bufs=2, space="PSUM") as psA,
            tc.tile_pool(name="psO", bufs=2, space="PSUM") as psO,
        ):
          for _rep in range(repeat):
            # ---- load inputs ----
            x_sb = sing.tile([CIN + 1, N], f32)
            for j in range(NBLK):
                nc.sync.dma_start(
                    out=x_sb[:, j * NB:(j + 1) * NB],
                    in_=x_d[:, j * NB:(j + 1) * NB],
                )
            wq_sb = sing.tile([CIN + 1, C], f32)
            wk_sb = sing.tile([CIN + 1, C], f32)
            wv_sb = sing.tile([CIN + 1, C + 1], f32)
            nc.sync.dma_start(out=wq_sb, in_=wq_d[:])
            nc.sync.dma_start(out=wk_sb, in_=wk_d[:])
            nc.sync.dma_start(out=wv_sb, in_=wv_d[:])
            ones_sb = sing.tile([1, C], f32)
            nc.vector.memset(ones_sb, 1.0)

            # ---- q, k = W_aug @ x_aug  (bias via ones row of x_aug) ----
            q_sb = sing.tile([C, N + 1], f32)
            k_sb = sing.tile([C, N], f32)
            qsum8 = sing.tile([C, 8], f32)
            ksum8 = sing.tile([C, 8], f32)
            for j in range(N // HALF):
                qp = psA.tile([C, HALF], f32, tag="S")
                nc.tensor.matmul(
                    qp, wq_sb, x_sb[:, j * HALF:(j + 1) * HALF],
                    start=True, stop=True,
                )
                nc.scalar.activation(
                    out=q_sb[:, j * HALF:(j + 1) * HALF], in_=qp,
                    func=AF.Copy, accum_out=qsum8[:, j:j + 1],
                )
                kp = psA.tile([C, HALF], f32, tag="S")
                nc.tensor.matmul(
                    kp, wk_sb, x_sb[:, j * HALF:(j + 1) * HALF],
                    start=True, stop=True,
                )
                nc.scalar.activation(
                    out=k_sb[:, j * HALF:(j + 1) * HALF], in_=kp,
                    func=AF.Copy, accum_out=ksum8[:, j:j + 1],
                )

            # ---- vT chunks ([m, c] layout) with ones column ----
            vT_sb = sing.tile([MB, MCH, C + 1], f32)
            for t in range(MCH):
                vp = psA.tile([MB, C + 1], f32, tag="S")
                nc.tensor.matmul(
                    vp, x_sb[:, t * MB:(t + 1) * MB], wv_sb,
                    start=True, stop=True,
                )
                nc.vector.tensor_copy(vT_sb[:, t, :], vp)

            # ---- means; whiten q, k; unary query column ----
            qsum = sing.tile([C, 1], f32)
            ksum = sing.tile([C, 1], f32)
            nc.vector.reduce_sum(qsum, qsum8, axis=AX.X)
            nc.vector.reduce_sum(ksum, ksum8, axis=AX.X)
            nqm = sing.tile([C, 1], f32)
            nkm = sing.tile([C, 1], f32)
            nc.vector.tensor_scalar_mul(nqm, qsum, -1.0 / N)
            nc.vector.tensor_scalar_mul(nkm, ksum, -1.0 / N)
            # unary column: 8 * q_mean (so SCALE * col . k' = q_mean . k')
            nc.scalar.mul(q_sb[:, N:N + 1], qsum, 8.0 / N)
            nc.vector.tensor_scalar_add(k_sb, k_sb, nkm)
            nc.vector.tensor_scalar_add(q_sb[:, 0:N], q_sb[:, 0:N], nqm)

            # ---- unary attention (single extra query column) ----
            su = psA.tile([MB, MCH], f32, tag="S")
            for t in range(MCH):
                nc.tensor.matmul(
                    su[:, t:t + 1], k_sb[:, t * MB:(t + 1) * MB],
                    q_sb[:, N:N + 1], start=True, stop=True,
                )
            eu = epool.tile([MB, MCH], f32, tag="E")
            nc.scalar.activation(out=eu, in_=su, func=AF.Exp, scale=SCALE)
            uacc = psO.tile([C + 1, 1], f32, tag="O")
            for t in range(MCH):
                nc.tensor.matmul(
                    uacc, vT_sb[:, t, :], eu[:, t:t + 1],
                    start=(t == 0), stop=(t == MCH - 1),
                )
            du = sing.tile([1, 1], f32)
            nc.vector.tensor_copy(du, uacc[C:C + 1, 0:1])
            recu = sing.tile([1, 1], f32)
            nc.vector.reciprocal(recu, du)
            bcu = psA.tile([C, 1], f32, tag="S")
            nc.tensor.matmul(bcu, ones_sb, recu, start=True, stop=True)
            ucp = sing.tile([C, 1], f32)
            nc.vector.tensor_copy(ucp, uacc[0:C, 0:1])
            u_sb = sing.tile([C, 1], f32)
            nc.vector.tensor_mul(u_sb, ucp, bcu)
            # x + unary term, broadcast along n
            xpu = sing.tile([C, N], f32)
            nc.vector.tensor_scalar_add(xpu, x_sb[0:C, :], u_sb)

            # ---- main attention loop ----
            for j in range(NBLK):
                o_ps = psO.tile([C + 1, NB], f32, tag="O")
                for t in range(MCH):
                    s_ps = psA.tile([MB, NB], f32, tag="S")
                    for h in range(NB // HALF):
                        nc.tensor.matmul(
                            s_ps[:, h * HALF:(h + 1) * HALF],
                            k_sb[:, t * MB:(t + 1) * MB],
                            q_sb[:, j * NB + h * HALF:j * NB + (h + 1) * HALF],
                            start=True, stop=True,
                        )
                    e_sb = epool.tile([MB, NB], f32, tag="E")
                    nc.scalar.activation(out=e_sb, in_=s_ps, func=AF.Exp,
                                         scale=SCALE)
                    for h in range(NB // HALF):
                        nc.tensor.matmul(
                            o_ps[:, h * HALF:(h + 1) * HALF],
                            vT_sb[:, t, :],
                            e_sb[:, h * HALF:(h + 1) * HALF],
                            start=(t == 0), stop=(t == MCH - 1),
                        )
                d_sb = dpool.tile([1, NB], f32, tag="d")
                nc.vector.tensor_copy(d_sb, o_ps[C:C + 1, :])
                rec = dpool.tile([1, NB], f32, tag="rec")
                nc.vector.reciprocal(rec, d_sb)
                bc_ps = psA.tile([C, NB], f32, tag="S")
                for h in range(NB // HALF):
                    nc.tensor.matmul(
                        bc_ps[:, h * HALF:(h + 1) * HALF], ones_sb,
                        rec[:, h * HALF:(h + 1) * HALF],
                        start=True, stop=True,
                    )
                bcs = opool.tile([C, NB], f32, tag="bcs")
                nc.vector.tensor_copy(bcs, bc_ps)
                o_sb = opool.tile([C, NB], f32, tag="o")
                nc.vector.tensor_mul(o_sb, o_ps[0:C, :], bcs)
                nc.vector.tensor_add(o_sb, o_sb, xpu[:, j * NB:(j + 1) * NB])
                nc.sync.dma_start(out=out_d[:, j * NB:(j + 1) * NB], in_=o_sb)

    if compat:
        _fix_walrus_compat(nc)
    return nc


def _fix_walrus_compat(nc):
    """Work around version skew between concourse and this walrus build.

    1. This walrus accepts at most ONE sync wait per instruction
       (setupSyncWait: "Too many sync wait commands").  Excess waits move
       to same-engine NOPs inserted immediately before the instruction —
       engine program order preserves the wait-before-execute semantics.
    2. EVENT_SEMAPHORE_RANGE_CLEAR (emitted by TileContext exit to reset
       tile semaphores) has a different ISA struct length in this walrus
       ("ISA wrong length").  Replace with one NOP per semaphore carrying
       a sem-wr-imm 0 update.
    """
    from concourse import mybir

    for f in nc.m.functions:
        for blk in f.blocks:
            new = []
            for inst in blk.instructions:
                si = inst.sync_info
                if (type(inst).__name__ == "InstISA"
                        and getattr(inst, "op_name", None)
                        == "EVENT_SEMAPHORE_RANGE_CLEAR"):
                    d = inst.ant_dict
                    first, last = d["range_first"], d["range_last"]
                    waits = list(si.on_wait) if si else []
                    for s in range(first, last + 1):
                        upd = mybir.SyncUpdate(
                            sync_type="semaphore", id=s,
                            ant_name=f"semreset_{s}",
                            update_mode="sem-wr-imm", update_value=0,
                            update_reg=None)
                        nop = mybir.InstNoOp(
                            name=f"semreset_{nc.next_id()}",
                            sync_info=mybir.SyncInfo(
                                on_wait=[waits.pop()] if waits else [],
                                on_update=[upd]),
                            bass_nofuse=True,
                            engine=inst.engine)
                        new.append(nop)
                    while waits:
                        nop = mybir.InstNoOp(
                            name=f"semreset_{nc.next_id()}",
                            sync_info=mybir.SyncInfo(
                                on_wait=[waits.pop()], on_update=[]),
                            bass_nofuse=True, engine=inst.engine)
                        new.insert(0, nop)
                    continue
                if si is not None and len(si.on_wait) > 1:
                    waits = list(si.on_wait)
                    excess, keep = waits[:-1], waits[-1:]
                    for w in excess:
                        nop = mybir.InstNoOp(
                            name=f"mwfix_{nc.next_id()}",
                            sync_info=mybir.SyncInfo(on_wait=[w], on_update=[]),
                            bass_nofuse=True,
                            engine=inst.engine)
                        new.append(nop)
                    inst.sync_info = mybir.SyncInfo(
                        on_wait=keep, on_update=list(si.on_update))
                new.append(inst)
            blk.instructions[:] = new


def _prep_inputs(x, wq, bq, wk, bk, wv, bv):
    """Host-side shard prep: per-core input maps (batch i -> core i)."""
    x = np.asarray(x, np.float32)
    wqT = np.concatenate([np.asarray(wq, np.float32).T,
                          np.asarray(bq, np.float32)[None, :]], 0)
    wkT = np.concatenate([np.asarray(wk, np.float32).T,
                          np.asarray(bk, np.float32)[None, :]], 0)
    wvT = np.zeros((CIN + 1, C + 1), np.float32)
    wvT[:CIN, :C] = np.asarray(wv, np.float32).T
    wvT[CIN, :C] = np.asarray(bv, np.float32)
    wvT[CIN, C] = 1.0
    ones = np.ones((1, N), np.float32)
    maps = []
    for i in range(B):
        xa = np.concatenate([x[i].reshape(CIN, N), ones], 0)
        maps.append({"x_aug": np.ascontiguousarray(xa),
                     "wqT_aug": wqT, "wkT_aug": wkT, "wvT_aug": wvT})
    return maps


def kernel(x, wq, bq, wk, bk, wv, bv):
    from concourse.bass_utils import run_bass_kernel_spmd

    if "nc" not in _CACHE:
        _CACHE["nc"] = _build()
    nc = _CACHE["nc"]
    in_maps = _prep_inputs(x, wq, bq, wk, bk, wv, bv)
    res = run_bass_kernel_spmd(nc, in_maps, list(range(B)))
    out = np.stack([res.results[i]["out"].reshape(C, H, W) for i in range(B)])
    return out.astype(np.float32)
